# revision 1
# baseline (speedup 1.0000x reference)
"""Trainium2 Bass kernel for nn_CorrBlockSingleScale (RAFT single-scale
correlation lookup), distributed over 8 NeuronCores.

  fmap1, fmap2: [1, 256, 64, 96] f32;  coords: [1, 2, 64, 96] f32; radius=4
  corr = einsum('bcm,bcn->bmn', f1, f2) / 16        -> [6144, 64, 96]
  out[q, i, j] = bilinear(corr[q], (cx_q + d_i, cy_q + d_j)),  d in -4..4
  output [1, 81, 64, 96] f32.

Structure exploited: the 9x9 sample offsets are integers, so all 81 samples
of a query share one fractional pair (fx, fy) -- the output is a separable
2x2-tap blend of a 10x10 patch of corr[q] anchored at
(floor(cx)-4, floor(cy)-4).

Distribution (no collectives): queries are sorted by floor(cy) on the host;
each core takes 768 contiguous sorted queries and therefore only needs a
narrow y-band (~19 of 64 rows) of the correlation target plane.  Per core:
  1. matmul f1_tile^T @ f2_band with K=256 split into bf16 hi/lo pairs
     (3 accumulating matmuls per K-half: hi*hi, hi*lo, lo*hi -- fp32-class
     accuracy at bf16 PE throughput).  Band columns are host-permuted to
     x-major order so each query's corr band lands transposed in DRAM.
  2. DMA the band to a per-tile DRAM scratch slot per query.
  3. indirect-DMA gather one contiguous window per query (the 10x10 patch
     spans 9*W_ROWS+10 elements in the x-major layout).
  4. blend the patch with host-folded bilinear weights + validity masks on
     the vector engine; DMA out [128, 81] rows.
Host post-pass inverse-permutes and transposes to the reference layout.
"""


import numpy as np

import concourse.bass as bass
import concourse.bacc as bacc
import concourse.mybir as mybir
import concourse.tile as tile
from concourse import bass_utils
from concourse.bass import ts

F32 = mybir.dt.float32
I32 = mybir.dt.int32

B, C, H, W = 1, 256, 64, 96
R = 4
K = 2 * R + 1          # 9
PK = K + 1             # 10 (patch side)
NQ = H * W             # 6144
NCORES = 8
QPC = NQ // NCORES     # 768
P = 128
NT = QPC // P          # 6 tiles per core
GUARD = 512            # head guard (window can start below the slot)
GUARD_TAIL = 1024      # tail guard (window can end past the last slot)


# --------------------------------------------------------------------------
# host-side preprocessing
# --------------------------------------------------------------------------

def host_preprocess(fmap1, fmap2, coords):
    """Returns (in_maps, order, NF)."""
    f1 = np.asarray(fmap1, np.float32).reshape(C, NQ)
    f2 = np.asarray(fmap2, np.float32).reshape(C, NQ)
    cx = np.asarray(coords, np.float32)[0, 0].reshape(NQ)
    cy = np.asarray(coords, np.float32)[0, 1].reshape(NQ)

    ix = np.floor(cx)
    iy = np.floor(cy)
    fx = cx - ix          # exact in fp32
    fy = cy - iy
    ixi = ix.astype(np.int64)
    iyi = iy.astype(np.int64)

    order = np.argsort(iyi, kind="stable")

    # uniform band width across cores
    w_req = 0
    for c in range(NCORES):
        qs = order[c * QPC:(c + 1) * QPC]
        w_req = max(w_req, int(iyi[qs].max() - iyi[qs].min()) + PK)
    W_ROWS = min(H, w_req)
    NF = W_ROWS * W

    in_maps = []
    for c in range(NCORES):
        qs = order[c * QPC:(c + 1) * QPC]
        miny = int(iyi[qs].min())
        r0 = int(np.clip(miny - R, 0, H - W_ROWS))

        f1s = f1[:, qs].reshape(2, P, QPC)
        # band columns reordered x-major (c*W_ROWS + r): the corr band then
        # lands in DRAM transposed per query, so a patch window spans only
        # 9*W_ROWS+10 elements instead of 9*96+10.
        f2w = f2[:, r0 * W: r0 * W + NF].reshape(C, W_ROWS, W)
        f2s = np.ascontiguousarray(f2w.transpose(0, 2, 1).reshape(2, P, NF))

        jy = iyi[qs]           # [768]
        jx = ixi[qs]
        a = np.arange(PK)      # [10]
        r_abs = jy[:, None] - R + a[None, :]            # patch row abs y
        # per-query window start (one gather offset per query)
        idx = (GUARD + (np.arange(QPC) % P) * NF
               + (jx - R) * W_ROWS + (jy - R - r0)).astype(np.int32)[:, None]

        bcol = np.arange(PK)
        mx = ((jx[:, None] - R + bcol[None, :] >= 0)
              & (jx[:, None] - R + bcol[None, :] <= W - 1))   # [768,10]
        my = (r_abs >= 0) & (r_abs <= H - 1)                  # [768,10]
        # transposed mask layout [q, b(x), a(y)]
        m2 = (mx[:, :, None] & my[:, None, :]).astype(np.float32)

        wx1 = fx[qs].astype(np.float32)
        wy1 = fy[qs].astype(np.float32)
        # inner (window-minor) axis is y -> inner mix uses wy, outer uses wx
        wts = np.stack([(1.0 - wy1), wy1,
                        (1.0 - wx1) / 16.0, wx1 / 16.0], axis=1).astype(np.float32)

        in_maps.append({
            "f1s": np.ascontiguousarray(f1s),
            "f2s": np.ascontiguousarray(f2s),
            "idx": idx,
            "m2": np.ascontiguousarray(m2.reshape(QPC, PK * PK)),
            "wts": np.ascontiguousarray(wts),
        })
    return in_maps, order, NF


def split_bf16_inputs(in_maps):
    """Replace f1s/f2s with bf16 hi/lo splits (for mm_dtype='bf16x3')."""
    import ml_dtypes
    bf16 = ml_dtypes.bfloat16
    out = []
    for m in in_maps:
        m = dict(m)
        for name in ("f1s", "f2s"):
            x = m.pop(name).astype(np.float32)
            hi = x.astype(bf16)
            lo = (x - hi.astype(np.float32)).astype(bf16)
            m[name + "h"] = hi
            m[name + "l"] = lo
        out.append(m)
    return out


def assemble_output(results, order):
    rows = np.concatenate([results[c]["out"] for c in range(NCORES)], axis=0)
    # device blend emits [dx, dy]-major, matching the reference's 81-axis
    # (delta[..., 0] is added to x and varies along the first grid axis)
    full = np.empty((K * K, NQ), np.float32)
    full[:, order] = rows.T
    return full.reshape(1, K * K, H, W)


# --------------------------------------------------------------------------
# device program
# --------------------------------------------------------------------------

def _body(tc, nc, aps, scr, NF, nchunks, mm_dtype=F32):
    idx, m2, wts, out = aps["idx"], aps["m2"], aps["wts"], aps["out"]
    bf3 = (mm_dtype == "bf16x3")
    import contextlib
    ctx = contextlib.ExitStack()
    with ctx:
        const = ctx.enter_context(tc.tile_pool(name="const", bufs=1))
        corr_pool = ctx.enter_context(tc.tile_pool(name="corr", bufs=2))
        psum_pool = ctx.enter_context(
            tc.tile_pool(name="ps", bufs=4, space="PSUM"))
        small = ctx.enter_context(tc.tile_pool(name="small", bufs=3))

        # resident inputs.  mm_list: (lhsT sbuf tile, rhs sbuf tile, k) per
        # accumulating matmul of one output chunk.
        if bf3:
            BF = mybir.dt.bfloat16
            f1bh = const.tile([P, 2 * QPC], BF)
            f1bl = const.tile([P, 2 * QPC], BF)
            f2bh0 = const.tile([P, NF], BF)
            f2bl0 = const.tile([P, NF], BF)
            f2bh1 = const.tile([P, NF], BF)
            f2bl1 = const.tile([P, NF], BF)
            for k in range(2):
                nc.sync.dma_start(f1bh[:, k * QPC:(k + 1) * QPC],
                                  aps["f1sh"][k])
                nc.sync.dma_start(f1bl[:, k * QPC:(k + 1) * QPC],
                                  aps["f1sl"][k])
            nc.sync.dma_start(f2bh0[:], aps["f2sh"][0])
            nc.sync.dma_start(f2bh1[:], aps["f2sh"][1])
            nc.sync.dma_start(f2bl0[:], aps["f2sl"][0])
            nc.sync.dma_start(f2bl1[:], aps["f2sl"][1])
            f2bh = [f2bh0, f2bh1]
            f2bl = [f2bl0, f2bl1]
            mm_list = [(f1bh, f2bh[0], 0), (f1bh, f2bh[1], 1),
                       (f1bh, f2bl[0], 0), (f1bl, f2bh[0], 0),
                       (f1bh, f2bl[1], 1), (f1bl, f2bh[1], 1)]
        else:
            f1b = const.tile([P, 2 * QPC], F32)
            nc.sync.dma_start(f1b[:, 0:QPC], aps["f1s"][0])
            nc.sync.dma_start(f1b[:, QPC:2 * QPC], aps["f1s"][1])
            f2b0 = const.tile([P, NF], F32)
            nc.sync.dma_start(f2b0[:], aps["f2s"][0])
            f2b1 = const.tile([P, NF], F32)
            nc.sync.dma_start(f2b1[:], aps["f2s"][1])
            f2b = [f2b0, f2b1]
            mm_list = [(f1b, f2b[0], 0), (f1b, f2b[1], 1)]

        idxb = const.tile([P, NT], I32)
        nc.sync.dma_start(idxb[:].rearrange("p (t a) -> p t a", a=1),
                          idx.rearrange("(t p) a -> p t a", p=P))
        m2b = const.tile([P, NT * PK * PK], F32)
        nc.sync.dma_start(m2b[:].rearrange("p (t a) -> p t a", a=PK * PK),
                          m2.rearrange("(t p) a -> p t a", p=P))
        wtsb = const.tile([P, NT * 4], F32)
        nc.sync.dma_start(wtsb[:].rearrange("p (t a) -> p t a", a=4),
                          wts.rearrange("(t p) a -> p t a", p=P))

        chunks = [(i * 512, min(512, NF - i * 512)) for i in range(nchunks)]

        # zero the scratch guard bands (a masked-out window row may read them;
        # uninitialized HBM could hold NaN and 0*NaN would poison the blend)
        zt = const.tile([1, GUARD_TAIL], F32)
        nc.vector.memset(zt[:], 0.0)
        for t in range(NT):
            g = scr[t].ap()[0:GUARD].rearrange("(p f) -> p f", p=1)
            nc.sync.dma_start(g, zt[:, 0:GUARD])
            g = scr[t].ap()[GUARD + P * NF:GUARD + P * NF + GUARD_TAIL] \
                .rearrange("(p f) -> p f", p=1)
            nc.sync.dma_start(g, zt[:])

        for t in range(NT):
            corr_sb = corr_pool.tile([P, NF], F32)
            for ci, (c0, cw) in enumerate(chunks):
                ps = psum_pool.tile([P, 512], F32, space="PSUM", tag="ps")
                for mi, (f1t, f2t, k) in enumerate(mm_list):
                    lhsT = f1t[:, k * QPC + t * P: k * QPC + (t + 1) * P]
                    rhs = f2t[:, c0:c0 + cw]
                    if not bf3 and mm_dtype != F32:
                        lhsT = lhsT.bitcast(mm_dtype)
                        rhs = rhs.bitcast(mm_dtype)
                    nc.tensor.matmul(
                        ps[:, :cw], lhsT=lhsT, rhs=rhs,
                        start=(mi == 0), stop=(mi == len(mm_list) - 1))
                # alternate PSUM->SBUF copies across ACT and DVE
                if ci % 2 == 0:
                    nc.scalar.copy(corr_sb[:, c0:c0 + cw], ps[:, :cw])
                else:
                    nc.vector.tensor_copy(corr_sb[:, c0:c0 + cw], ps[:, :cw])

            dst = scr[t].ap()[GUARD:GUARD + P * NF] \
                .rearrange("(p f) -> p f", p=P)
            nc.sync.dma_start(dst, corr_sb[:])

            wrows = NF // W
            win = (PK - 1) * wrows + PK
            pt = small.tile([P, PK * wrows], F32, tag="pt")
            src = scr[t].ap().rearrange("(n o) -> n o", o=1)
            nc.gpsimd.indirect_dma_start(
                out=pt[:, 0:win], out_offset=None, in_=src,
                in_offset=bass.IndirectOffsetOnAxis(
                    ap=idxb[:, t:t + 1], axis=0))
            # patch view: x-strips at stride wrows inside the gathered window
            ptv = pt[:].rearrange("p (b r) -> p b r", r=wrows)[:, :, 0:PK]

            pm = small.tile([P, PK * PK], F32, tag="pm")
            nc.vector.tensor_tensor(
                pm[:].rearrange("p (a b) -> p a b", b=PK), ptv,
                m2b[:, ts(t, PK * PK)].rearrange("p (a b) -> p a b", b=PK),
                op=mybir.AluOpType.mult)
            pm3 = pm[:].rearrange("p (a b) -> p a b", b=PK)

            t1 = small.tile([P, PK * K], F32, tag="t1")
            t13 = t1[:].rearrange("p (a b) -> p a b", b=K)
            nc.vector.tensor_scalar_mul(
                t13, pm3[:, :, 1:PK], wtsb[:, 4 * t + 1: 4 * t + 2])
            cm = small.tile([P, PK * K], F32, tag="cm")
            cm3 = cm[:].rearrange("p (a b) -> p a b", b=K)
            nc.vector.scalar_tensor_tensor(
                cm3, pm3[:, :, 0:K], wtsb[:, 4 * t: 4 * t + 1], t13,
                op0=mybir.AluOpType.mult, op1=mybir.AluOpType.add)

            t2 = small.tile([P, K * K], F32, tag="t2")
            t23 = t2[:].rearrange("p (a b) -> p a b", b=K)
            nc.vector.tensor_scalar_mul(
                t23, cm3[:, 1:PK, :], wtsb[:, 4 * t + 3: 4 * t + 4])
            ot = small.tile([P, K * K], F32, tag="ot")
            ot3 = ot[:].rearrange("p (a b) -> p a b", b=K)
            nc.vector.scalar_tensor_tensor(
                ot3, cm3[:, 0:K, :], wtsb[:, 4 * t + 2: 4 * t + 3], t23,
                op0=mybir.AluOpType.mult, op1=mybir.AluOpType.add)

            nc.sync.dma_start(out[ts(t, P), :], ot[:])


def build_program(NF, rep=1, mm_dtype=F32):
    """rep>1 wraps the body in a For_i loop (for wall-clock timing)."""
    nchunks = (NF + 511) // 512
    nc = bacc.Bacc("TRN2", target_bir_lowering=False, debug=False,
                   num_devices=NCORES)
    aps = {}
    if mm_dtype == "bf16x3":
        BF = mybir.dt.bfloat16
        for nm in ("f1sh", "f1sl"):
            aps[nm] = nc.dram_tensor(nm, [2, P, QPC], BF,
                                     kind="ExternalInput").ap()
        for nm in ("f2sh", "f2sl"):
            aps[nm] = nc.dram_tensor(nm, [2, P, NF], BF,
                                     kind="ExternalInput").ap()
    else:
        aps["f1s"] = nc.dram_tensor("f1s", [2, P, QPC], F32,
                                    kind="ExternalInput").ap()
        aps["f2s"] = nc.dram_tensor("f2s", [2, P, NF], F32,
                                    kind="ExternalInput").ap()
    aps["idx"] = nc.dram_tensor("idx", [QPC, 1], I32,
                                kind="ExternalInput").ap()
    aps["m2"] = nc.dram_tensor("m2", [QPC, PK * PK], F32,
                               kind="ExternalInput").ap()
    aps["wts"] = nc.dram_tensor("wts", [QPC, 4], F32,
                                kind="ExternalInput").ap()
    aps["out"] = nc.dram_tensor("out", [QPC, K * K], F32,
                                kind="ExternalOutput").ap()
    scr = [nc.dram_tensor(f"scr{t}", [GUARD + P * NF + GUARD_TAIL], F32)
           for t in range(NT)]

    with tile.TileContext(nc) as tc:
        if rep == 1:
            _body(tc, nc, aps, scr, NF, nchunks, mm_dtype)
        else:
            with tc.For_i(0, rep):
                _body(tc, nc, aps, scr, NF, nchunks, mm_dtype)
    nc.compile()
    return nc


_PROGRAMS = {}


def kernel(fmap1, fmap2, coords, radius):
    assert int(radius) == R, f"kernel hardcodes radius=4, got {radius}"
    in_maps, order, NF = host_preprocess(fmap1, fmap2, coords)
    in_maps = split_bf16_inputs(in_maps)
    nc = _PROGRAMS.get(NF)
    if nc is None:
        nc = _PROGRAMS[NF] = build_program(NF, mm_dtype="bf16x3")
    last_err = None
    for _ in range(3):  # the remote compile hook occasionally flakes
        try:
            res = bass_utils.run_bass_kernel_spmd(
                nc, in_maps, core_ids=list(range(NCORES)))
            return assemble_output(res.results, order)
        except Exception as e:  # noqa: BLE001
            last_err = e
    raise last_err



# revision 10
# speedup vs baseline: 1.6756x; 1.6756x over previous
"""Trainium2 Bass kernel for nn_CorrBlockSingleScale (RAFT single-scale
correlation lookup), distributed over 8 NeuronCores.

  fmap1, fmap2: [1, 256, 64, 96] f32;  coords: [1, 2, 64, 96] f32; radius=4
  corr = einsum('bcm,bcn->bmn', f1, f2) / 16        -> [6144, 64, 96]
  out[q, i, j] = bilinear(corr[q], (cx_q + d_i, cy_q + d_j)),  d in -4..4
  output [1, 81, 64, 96] f32.

v2 design (vs the bf16x3 full-band baseline):
  * Queries are sorted by floor(cx); each core owns 768 contiguous sorted
    queries -> a narrow x-band (~22 of 96 cols) of the target frame.
  * Within a core, queries are assigned to NT static y-slabs (slab t's
    patch window = band rows [t*S-4, t*S-4+BH)), each slab holding <= 128
    queries (padded with duplicates).  Static windows mean the matmul rhs
    slice offsets are compile-time constants shared by all 8 SPMD cores.
  * The band is zero-padded outside the image, which reproduces
    grid_sample's padding_mode='zeros' exactly -> no validity masks.
  * Per slab: one accumulation group of 2 bf16 matmuls (K=256 split in
    half) computes corr for 128 queries x (BH*BW~400) window columns.
    Plain bf16 is ~1e-3 relative error; the gate is 2e-2.
  * corr -> SBUF (bf16) -> DRAM scratch -> indirect-DMA gather of one
    contiguous 9*BW+10 window per query (x-shifted per query) -> 4-op
    separable bilinear blend on DVE with per-partition scalars -> out.
"""

import numpy as np
import ml_dtypes

import concourse.bass as bass
import concourse.bacc as bacc
import concourse.mybir as mybir
import concourse.tile as tile
from concourse import bass_utils

F32 = mybir.dt.float32
I32 = mybir.dt.int32
BF = mybir.dt.bfloat16
NPBF = ml_dtypes.bfloat16

B, C, H, W = 1, 256, 64, 96
R = 4
K = 2 * R + 1          # 9
PK = K + 1             # 10 (patch side)
NQ = H * W             # 6144
NCORES = 8
QPC = NQ // NCORES     # 768
P = 128


# --------------------------------------------------------------------------
# host-side preprocessing
# --------------------------------------------------------------------------

def _assign_slabs(yv, NT, S, COV, cap=P):
    """Greedy earliest-eligible-slab assignment of queries (by iy) to NT
    static y-slabs; slab t accepts iy in [t*S, t*S+COV). Returns per-slab
    index lists into yv's order, or None on overflow."""
    slots = [[] for _ in range(NT)]
    order = np.argsort(yv, kind="stable")
    for i in order:
        v = int(yv[i])
        tmin = max(0, -(-(v - COV + 1) // S))
        tmax = min(NT - 1, v // S)
        for t in range(tmin, tmax + 1):
            if len(slots[t]) < cap:
                slots[t].append(i)
                break
        else:
            return None
    return slots


def host_preprocess(fmap1, fmap2, coords):
    f1 = np.asarray(fmap1, np.float32).reshape(C, NQ)
    f2 = np.asarray(fmap2, np.float32).reshape(C, H, W)
    cx = np.asarray(coords, np.float32)[0, 0].reshape(NQ)
    cy = np.asarray(coords, np.float32)[0, 1].reshape(NQ)
    ix = np.floor(cx).astype(np.int64)
    iy = np.floor(cy).astype(np.int64)
    fx = (cx - ix).astype(np.float32)
    fy = (cy - iy).astype(np.float32)

    order_x = np.argsort(ix, kind="stable")
    BW = PK + max(
        int(ix[order_x[c * QPC:(c + 1) * QPC]].max()
            - ix[order_x[c * QPC:(c + 1) * QPC]].min())
        for c in range(NCORES))

    # smallest static-slab geometry that fits this input
    for NT, S, COV in [(8, 8, 9), (8, 8, 10), (9, 7, 9), (10, 6, 10),
                       (12, 5, 10), (16, 4, 7)]:
        if (NT - 1) * S + COV < H:
            continue
        percore = []
        for c in range(NCORES):
            qs = order_x[c * QPC:(c + 1) * QPC]
            slabs = _assign_slabs(iy[qs], NT, S, COV)
            if slabs is None:
                break
            percore.append((qs, slabs))
        else:
            break
    else:
        raise AssertionError("no slab geometry fits")
    BH = COV + PK - 1
    N_t = BH * BW
    assert N_t <= 512, (BH, BW)

    nrows = (NT - 1) * S + BH        # padded band rows [-R, -R+nrows)
    NFB = nrows * BW
    WINPAD = PK * BW
    WIN = (PK - 1) * BW + PK + 1     # blend views touch (PK-1)*BW + PK

    in_maps = []
    qmeta = []
    for c in range(NCORES):
        qs, slabs = percore[c]
        bx0 = int(ix[qs].min()) - R

        # ---- f1 + f2 band, one bf16 array [2, 128, NT*128 + NFB] ----
        QF = NT * P
        fb = np.zeros((2, P, QF + NFB), NPBF)
        # band rows are image rows [-R, nrows-R), zero-padded
        y0, y1 = R, min(nrows, H + R)          # valid storage rows
        xs = max(0, -bx0)                       # valid storage cols
        xe = min(BW, W - bx0)
        band = np.zeros((C, nrows, BW), np.float32)
        band[:, y0:y1, xs:xe] = f2[:, y0 - R:y1 - R, bx0 + xs:bx0 + xe]
        fb[:, :, QF:] = band.reshape(2, P, NFB).astype(NPBF)

        # slab-ordered query list, padded to P per slab
        qlists = []
        valid = []
        for t in range(NT):
            sl = [int(qs[i]) for i in slabs[t]]
            valid.append(len(sl))
            sl = sl + [sl[0] if sl else int(qs[0])] * (P - len(sl))
            qlists.append(sl)
        qflat = np.array(qlists).reshape(NT * P)    # [NT*128]
        # f1 tile t = cols [t*128, (t+1)*128)
        fb[:, :, :QF] = f1[:, qflat].reshape(2, P, QF).astype(NPBF)

        # ---- idx [128, NT] then wts [128, NT*4], packed as one i32 array ----
        iw = np.zeros((P, 5 * NT), np.int32)
        for t in range(NT):
            nv = valid[t]
            ql = np.array(qlists[t])
            dy = np.clip(iy[ql] - (t * S), 0, BH - PK)  # window row offset
            dx = np.clip(ix[ql] - R - bx0, 0, BW - PK)
            assert (iy[ql][:nv] - t * S >= 0).all()
            assert (iy[ql][:nv] - t * S <= BH - PK).all()
            iw[:, t] = (t * P + np.arange(P)) * N_t + dy * BW + dx
            wx0 = (1.0 - fx[ql]) / 16.0
            wx1 = fx[ql] / 16.0
            wy0 = 1.0 - fy[ql]
            wy1 = fy[ql]
            for k, w in enumerate((wx0, wx1, wy0, wy1)):
                iw[:, NT + 4 * t + k] = w.astype(np.float32).view(np.int32)

        in_maps.append({
            "fb": fb,
            "iw": np.ascontiguousarray(iw),
        })
        qmeta.append((qlists, valid))

    g = dict(BW=BW, BH=BH, NT=NT, S=S, N_t=N_t, NFB=NFB, nrows=nrows,
             WINPAD=WINPAD, WIN=WIN, QF=NT * P)
    return in_maps, qmeta, g


def assemble_output(results, qmeta, g):
    NT = g["NT"]
    full = np.empty((K * K, NQ), np.float32)
    for c in range(NCORES):
        rows = np.asarray(results[c]["out"], np.float32)   # [128, NT*90]
        rows = rows.reshape(P, NT, K, PK)[:, :, :, :K]     # [p, t, dy, dx]
        qlists, valid = qmeta[c]
        for t in range(NT):
            nv = valid[t]
            if nv == 0:
                continue
            qv = np.array(qlists[t][:nv])
            # device emits [dy, dx]; reference 81-axis is [dx, dy]
            full[:, qv] = rows[:nv, t].transpose(0, 2, 1).reshape(nv, 81).T
    return full.reshape(1, K * K, H, W)


# --------------------------------------------------------------------------
# device program
# --------------------------------------------------------------------------

def _body(tc, nc, aps, scr, g):
    NT, N_t, NFB, BW, BH = g["NT"], g["N_t"], g["NFB"], g["BW"], g["BH"]
    WINPAD, WIN, S = g["WINPAD"], g["WIN"], g["S"]
    QF = g["QF"]
    FBW = QF + NFB                       # free width per k-half of fb
    NG = 2                               # scratch/gather groups
    TPG = NT // NG                       # tiles per group
    import contextlib
    ctx = contextlib.ExitStack()
    with ctx:
        const = ctx.enter_context(tc.tile_pool(name="const", bufs=1))
        psum_pool = ctx.enter_context(
            tc.tile_pool(name="ps", bufs=4, space="PSUM"))
        pt_pool = ctx.enter_context(tc.tile_pool(name="pt", bufs=2))
        small = ctx.enter_context(tc.tile_pool(name="small", bufs=4))

        fb = const.tile([P, 2 * FBW], BF)
        fbv = fb[:].rearrange("p (k f) -> p k f", k=2)
        # split the band load: f1 + slabs [0, TPG) rows, then the rest
        r1 = min(nrows_needed := (TPG - 1) * S + BH, g["nrows"]) * BW
        nc.sync.dma_start(
            fbv[:, :, 0:QF + r1],
            aps["fb"][:, :, 0:QF + r1].rearrange("k p f -> p k f"))
        nc.sync.dma_start(
            fbv[:, :, QF + r1:],
            aps["fb"][:, :, QF + r1:].rearrange("k p f -> p k f"))

        iw = const.tile([P, NT * 5], I32)
        nc.sync.dma_start(iw[:], aps["iw"])
        idxv = iw[:, 0:NT]
        wts = iw[:, NT:5 * NT].bitcast(F32)   # [p, 4*NT]: (wx0,wx1,wy0,wy1)

        corr_sb = const.tile([P, NT * N_t], BF)
        out_sb = const.tile([P, NT * K * PK], BF)

        for grp in range(NG):
            t0, t1 = grp * TPG, (grp + 1) * TPG
            for t in range(t0, t1):
                ps = psum_pool.tile([P, N_t], F32, space="PSUM", tag="ps")
                for kh in range(2):
                    lhsT = fb[:, kh * FBW + t * P: kh * FBW + (t + 1) * P]
                    rhs = fb[:, kh * FBW + QF + t * S * BW:
                             kh * FBW + QF + t * S * BW + N_t]
                    nc.tensor.matmul(ps[:], lhsT=lhsT, rhs=rhs,
                                     start=(kh == 0), stop=(kh == 1))
                dst = corr_sb[:, t * N_t:(t + 1) * N_t]
                if t % 4 != 0:                    # ACT takes 3 of 4 copies
                    nc.scalar.copy(dst, ps[:])
                else:
                    nc.vector.tensor_copy(dst, ps[:])

            # scratch write: slot (t*128+p) holds query (t, p)'s window row
            dst = scr.ap()[t0 * P * N_t:t1 * P * N_t] \
                .rearrange("(t p f) -> p t f", t=TPG, p=P)
            nc.sync.dma_start(
                dst, corr_sb[:, t0 * N_t:t1 * N_t]
                .rearrange("p (t f) -> p t f", t=TPG))

            # gather one contiguous window per query (one call per slab:
            # the indirect-DMA offset ap only supports one offset/partition)
            pt = pt_pool.tile([P, TPG * WINPAD], BF, tag="pt")
            src = scr.ap().rearrange("(n o) -> n o", o=1)
            for t in range(t0, t1):
                tl = t - t0
                nc.gpsimd.indirect_dma_start(
                    out=pt[:, tl * WINPAD:tl * WINPAD + WIN],
                    out_offset=None, in_=src,
                    in_offset=bass.IndirectOffsetOnAxis(
                        ap=idxv[:, t:t + 1], axis=0))

            for t in range(t0, t1):
                tl = t - t0
                p3 = pt[:, tl * WINPAD:(tl + 1) * WINPAD] \
                    .rearrange("p (a w) -> p a w", w=BW)
                m = small.tile([P, PK * PK], BF, tag="m")
                m3 = m[:].rearrange("p (a b) -> p a b", b=PK)
                nc.vector.tensor_scalar_mul(
                    m3, p3[:, :, 1:PK + 1], wts[:, 4 * t + 1:4 * t + 2])
                tx = small.tile([P, PK * PK], BF, tag="tx")
                tx3 = tx[:].rearrange("p (a b) -> p a b", b=PK)
                nc.vector.scalar_tensor_tensor(
                    tx3, p3[:, :, 0:PK], wts[:, 4 * t:4 * t + 1], m3,
                    op0=mybir.AluOpType.mult, op1=mybir.AluOpType.add)

                m2 = small.tile([P, K * PK], BF, tag="m2")
                m23 = m2[:].rearrange("p (a b) -> p a b", b=PK)
                nc.vector.tensor_scalar_mul(
                    m23, tx3[:, 1:PK, :], wts[:, 4 * t + 3:4 * t + 4])
                ot3 = out_sb[:, t * K * PK:(t + 1) * K * PK] \
                    .rearrange("p (a b) -> p a b", b=PK)
                nc.vector.scalar_tensor_tensor(
                    ot3, tx3[:, 0:K, :], wts[:, 4 * t + 2:4 * t + 3], m23,
                    op0=mybir.AluOpType.mult, op1=mybir.AluOpType.add)

        nc.sync.dma_start(aps["out"], out_sb[:])


def build_program(g, rep=1):
    nc = bacc.Bacc("TRN2", target_bir_lowering=False, debug=False,
                   num_devices=NCORES)
    NT, N_t, NFB = g["NT"], g["N_t"], g["NFB"]
    aps = {
        "fb": nc.dram_tensor("fb", [2, P, g["QF"] + NFB], BF,
                             kind="ExternalInput").ap(),
        "iw": nc.dram_tensor("iw", [P, NT * 5], I32,
                             kind="ExternalInput").ap(),
        "out": nc.dram_tensor("out", [P, NT * K * PK], BF,
                              kind="ExternalOutput").ap(),
    }
    # +64: the gathered window over-reads up to WIN-(N_t-win_off) ~ 1 elem
    # past the last slot; keep it in-bounds.
    scr = nc.dram_tensor("scr", [NT * P * N_t + 64], BF)

    with tile.TileContext(nc) as tc:
        if rep == 1:
            _body(tc, nc, aps, scr, g)
        else:
            with tc.For_i(0, rep):
                _body(tc, nc, aps, scr, g)
    nc.compile()
    return nc


_PROGRAMS = {}


def kernel(fmap1, fmap2, coords, radius):
    assert int(radius) == R, f"kernel hardcodes radius=4, got {radius}"
    in_maps, qmeta, g = host_preprocess(fmap1, fmap2, coords)
    key = (g["BW"], g["BH"], g["NT"])
    nc = _PROGRAMS.get(key)
    if nc is None:
        nc = _PROGRAMS[key] = build_program(g)
    last_err = None
    for _ in range(3):  # the remote compile hook occasionally flakes
        try:
            res = bass_utils.run_bass_kernel_spmd(
                nc, in_maps, core_ids=list(range(NCORES)))
            return assemble_output(res.results, qmeta, g)
        except Exception as e:  # noqa: BLE001
            last_err = e
    raise last_err
